# revision 7
# baseline (speedup 1.0000x reference)
"""Trainium2 Bass kernel for nn_BaselineAttention_36172214567310.

Reference computation (note the einsum 'bhqk,bhkd->bhkd' bug: the "attention
output" is v scaled by the column-sums of the softmax matrix):

    qkv = x @ w_qkv                       # [b, s, 3*H*D]
    q, k, v = split(qkv)                  # per head
    P = softmax(q @ k^T / sqrt(D))        # [q, k] rows sum to 1
    colsum[k] = sum_q P[q, k]
    values[k, :] = v[k, :] * colsum_h[k]
    out = values @ w_o

Sharding: 8 cores = 2 batches x 4 head-groups (4 heads each).

Per-core pipeline (the ACT exp stream is the pole: 64 x [128,2048]
ACTIVATEs = ~128us):
  - Q/K projection in fp8 DoubleRow (K=256 per matmul), V in bf16.
  - Scores bf16, K=64, head pairs A/B row-tiled (lhsT base partitions 0/64)
    so both heads' score matmuls run concurrently in the PE array.
  - exp on ACT: [128,2048] chunks, fp8e4 E output in pair-interleaved layout
    [128,2,2048], with fused rowsum (accum_out).
  - colsum matvec: fp8 DoubleRow over qc-pairs (K=256), weights = 1/rowsum
    (fp8e5, replicated 64 stationary cols), A/B col-tiled; output borrows
    the B score PSUM tile and is flushed into an SBUF f32 accumulator.
  - v *= colsum, then out = values @ w_o per head-pair; pair partials summed
    in SBUF, one f32 DMA out. Host sums the 4 group partials per batch.

PSUM = exactly two [128,2048] f32 score tiles (8 banks, double buffered).
All other PE work (projection leftovers, phase-4 matmuls, matvec) borrows
the score tiles in the windows between an exp finishing and the next score
refill; the Tile framework's subtile dependency tracking serializes each
borrow against the surrounding reads/writes.
"""

import sys

sys.path.insert(0, "/opt/trn_rl_repo")

import numpy as np

B, S, HIDDEN = 2, 2048, 1024
NH, HD = 16, 64
N_CORES = 8
P = 128
QC = S // P          # 16 q chunks of 128
NPAIR = QC // 2      # 8 qc-pairs for the DoubleRow matvec

_CACHE = {}


def _build():
    if "nc" in _CACHE:
        return _CACHE["nc"]

    import concourse.mybir as mybir
    import concourse.tile as tile
    from concourse import bacc

    F32 = mybir.dt.float32
    BF16 = mybir.dt.bfloat16
    F8E4 = mybir.dt.float8e4
    F8E5 = mybir.dt.float8e5
    EXP = mybir.ActivationFunctionType.Exp
    ADD = mybir.AluOpType.add
    MULT = mybir.AluOpType.mult
    DR = mybir.MatmulPerfMode.DoubleRow

    nc = bacc.Bacc()
    xt8_d = nc.declare_dram_parameter("xt8", [P, 8, S], F8E4, isOutput=False)
    xtb_d = nc.declare_dram_parameter("xtb", [P, 8, S], BF16, isOutput=False)
    wqk8_d = nc.declare_dram_parameter("wqk8", [P, 8, 512], F8E4, isOutput=False)
    wvb_d = nc.declare_dram_parameter("wvb", [P, 8, 256], BF16, isOutput=False)
    wo_d = nc.declare_dram_parameter("wo", [256, HIDDEN], BF16, isOutput=False)
    out_d = nc.declare_dram_parameter("out", [S, HIDDEN], F32, isOutput=True)

    with tile.TileContext(nc) as tc:
        with tc.tile_pool(name="sb", bufs=1) as sb, \
             tc.tile_pool(name="ps", bufs=1, space="PSUM") as ps:

            # ---- persistent SBUF tiles ----
            qt = [sb.tile([P, S], BF16, name=f"qt{g}") for g in range(2)]
            kt = [sb.tile([P, S], BF16, name=f"kt{g}") for g in range(2)]
            vt = [sb.tile([P, S], BF16, name=f"vt{g}") for g in range(2)]
            xt8 = sb.tile([P, 8, S], F8E4, name="xt8")
            xtb = sb.tile([P, 8, S], BF16, name="xtb")
            wqk8 = sb.tile([P, 8, 512], F8E4, name="wqk8")
            wvb = sb.tile([P, 8, 256], BF16, name="wvb")
            wo_t = [sb.tile([P, HIDDEN], BF16, name=f"wo{g}") for g in range(2)]
            # E pairs + rowsums; ping-pong over qc-pair parity, per head
            e_t = [[sb.tile([P, 2, S], F8E4, name=f"e{h}{i}") for i in range(2)]
                   for h in range(2)]
            r_t = [[sb.tile([P, 2], F32, name=f"r{h}{i}") for i in range(2)]
                   for h in range(2)]
            rr_t = [[sb.tile([P, 1], F32, name=f"rr{h}{i}") for i in range(2)]
                    for h in range(2)]
            wr_t = [[sb.tile([P, 64], F8E5, name=f"wr{h}{i}") for i in range(2)]
                    for h in range(2)]
            acc = sb.tile([P, S], F32, name="acc")
            out_sb = sb.tile([P, QC, HIDDEN], F32, name="out_sb")
            warm = sb.tile([P, 1], F32, name="warm")

            # PSUM: two [128, 2048] tiles = all 8 banks
            T = [ps.tile([P, S], F32, name=f"T{i}") for i in range(2)]

            # ---- exp table preload (overlaps input DMA) ----
            nc.vector.memset(warm, 0.0)
            nc.scalar.activation(warm, warm, EXP)

            # ---- input DMA (Q/K path first: it gates the pipeline) ----
            nc.sync.dma_start(out=wqk8, in_=wqk8_d[:, :, :])
            nc.sync.dma_start(out=xt8, in_=xt8_d[:, :, :])
            for g in range(2):
                nc.sync.dma_start(out=wo_t[g], in_=wo_d[g * P:(g + 1) * P, :])
            nc.sync.dma_start(out=wvb, in_=wvb_d[:, :, :])
            nc.sync.dma_start(out=xtb, in_=xtb_d[:, :, :])

            # wqk8 column layout: [Q01, Q23, K01, K23] x 128
            def qk_group(mcol, dst, nt, slot):
                """fp8 DoubleRow projection group: 4 K=256 matmuls into a
                borrowed 512-col psum chunk, then f32->bf16 copy to dst."""
                pt = T[slot // 4][:, (slot % 4) * 512:(slot % 4 + 1) * 512]
                for j in range(4):
                    nc.tensor.matmul(
                        pt, wqk8[:, 2 * j:2 * j + 2, mcol * P:(mcol + 1) * P],
                        xt8[:, 2 * j:2 * j + 2, nt * 512:(nt + 1) * 512],
                        start=(j == 0), stop=(j == 3), perf_mode=DR)
                nc.vector.tensor_copy(
                    out=dst[:, nt * 512:(nt + 1) * 512], in_=pt)

            def v_group(mc, nt, half, slot):
                """bf16 V projection subgroup: 8 K=128 matmuls, N=256."""
                pt = T[slot // 4][:, (slot % 4) * 512:(slot % 4) * 512 + 256]
                lo = nt * 512 + half * 256
                for kc in range(8):
                    nc.tensor.matmul(
                        pt, wvb[:, kc, mc * P:(mc + 1) * P],
                        xtb[:, kc, lo:lo + 256],
                        start=(kc == 0), stop=(kc == 7))
                nc.vector.tensor_copy(out=vt[mc][:, lo:lo + 256], in_=pt)

            def ph4_group(g, sc, add_into, slot):
                """Phase 4 chunk: out[sc*128:(sc+1)*128, :] partial for head
                pair g. slot: 0/1 -> T[0] half, 2/3 -> T[1] half."""
                pt = T[slot // 2][:, (slot % 2) * 1024:(slot % 2 + 1) * 1024]
                for nh in range(2):
                    nc.tensor.matmul(
                        pt[:, nh * 512:(nh + 1) * 512],
                        vt[g][:, sc * P:(sc + 1) * P],
                        wo_t[g][:, nh * 512:(nh + 1) * 512],
                        start=True, stop=True)
                sl = out_sb[:, sc, :]
                if add_into:
                    nc.vector.tensor_tensor(sl, sl, pt, ADD)
                else:
                    nc.vector.tensor_copy(out=sl, in_=pt)

            def emit_scores(g, h, qc, tdst):
                """4 N=512 matmuls for head h (A=0/B=1) of pair g, chunk qc.
                K=64: head A uses PE rows 0-63, head B rows 64-127."""
                bp = h * 64
                for n in range(4):
                    nc.tensor.matmul(
                        tdst[:, n * 512:(n + 1) * 512],
                        qt[g][bp:bp + 64, qc * P:(qc + 1) * P],
                        kt[g][bp:bp + 64, n * 512:(n + 1) * 512],
                        start=True, stop=True)

            def emit_exp(g, h, t, tsrc):
                j, ko = divmod(t, 2)
                nc.scalar.activation(
                    e_t[h][j % 2][:, ko, :], tsrc, EXP, scale=0.125,
                    accum_out=r_t[h][j % 2][:, ko:ko + 1])

            def emit_wr(h, t):
                """1/rowsum weights for chunk t of head h: fp8e5, replicated
                across the 64 stationary columns."""
                j, ko = divmod(t, 2)
                nc.vector.reciprocal(rr_t[h][t % 2],
                                     r_t[h][j % 2][:, ko:ko + 1])
                nc.vector.tensor_copy(
                    out=wr_t[h][t % 2],
                    in_=rr_t[h][t % 2].to_broadcast([P, 64]))

            def emit_mv(t, first):
                """Colsum matvec for chunk t (plain fp8, K=128), both heads
                col-tiled (A -> psum partitions 0-63, B -> 64-127) into the
                borrowed T[1]; flush each 512-chunk into acc right away so
                the next scores-B refill is only gated chunk by chunk."""
                j, ko = divmod(t, 2)
                for c in range(4):
                    cs = slice(c * 512, (c + 1) * 512)
                    for h in range(2):
                        nc.tensor.matmul(
                            T[1][h * 64:h * 64 + 64, cs],
                            wr_t[h][t % 2],
                            e_t[h][j % 2][:, ko, cs],
                            start=True, stop=True)
                    if first:
                        nc.vector.tensor_copy(out=acc[:, cs], in_=T[1][:, cs])
                    else:
                        nc.vector.tensor_tensor(
                            acc[:, cs], acc[:, cs], T[1][:, cs], ADD)

            # ---- lead-in: K01 then Q01 projection (gates first scores) ----
            for i, mcol in enumerate((2, 0)):
                dst = kt[0] if mcol == 2 else qt[0]
                for nt in range(4):
                    qk_group(mcol, dst, nt, slot=i * 4 + nt)

            # Borrowed-window work queues (consumed one per iteration, using
            # the just-freed chunk 3 of T[0]).
            # pair-0 iterations t=0..15: V01 subgroups, then Q23/K23 proj.
            winA0 = [(v_group, (0, nt, hf)) for nt in range(4) for hf in range(2)]
            winA0 += [(qk_group, (mcol, kt[1] if mcol == 3 else qt[1], nt))
                      for mcol in (3, 1) for nt in range(4)]
            # pair-1 iterations t=17..31: V23 subgroups, then pair-0 phase 4.
            winA1 = [(v_group, (1, nt, hf)) for nt in range(4) for hf in range(2)]
            winA1 += [(ph4_group, (0, sc, False)) for sc in range(QC)]

            # ---- main loop: flat chunk stream t = g*16 + qc ----
            for t in range(2 * QC):
                g, qc = divmod(t, QC)
                # scores A -> T[0]
                emit_scores(g, 0, qc, T[0])
                # matvec for chunk t-2 (its weights were built last iter)
                if t >= 2:
                    emit_mv(t - 2, first=((t - 2) % QC == 0))
                if t == QC + 1:
                    # pair-0 colsums complete (flush of chunk 15 above)
                    nc.vector.tensor_tensor(vt[0], vt[0], acc, MULT)
                # scores B -> T[1]
                emit_scores(g, 1, qc, T[1])
                # exps (the pacing stream)
                emit_exp(g, 0, t, T[0])
                emit_exp(g, 1, t, T[1])
                # 1/rowsum matvec weights for the previous chunk
                if t >= 1:
                    for h in range(2):
                        emit_wr(h, t - 1)
                # borrowed-window work
                if g == 0:
                    if winA0:
                        fn, args = winA0.pop(0)
                        fn(*args, slot=3)
                elif t >= QC + 1 and winA1:
                    fn, args = winA1.pop(0)
                    fn(*args, slot=3 if fn is v_group else 1)

            # ---- tail ----
            for h in range(2):
                emit_wr(h, 2 * QC - 1)
            emit_mv(2 * QC - 2, first=False)
            emit_mv(2 * QC - 1, first=False)
            nc.vector.tensor_tensor(vt[1], vt[1], acc, MULT)
            for fn, args in winA1:                    # ph4 pair-0 leftovers
                fn(*args, slot=0)
            for sc in range(QC):
                ph4_group(1, sc, True, slot=2 + sc % 2)
                nc.sync.dma_start(out=out_d[sc * P:(sc + 1) * P, :],
                                  in_=out_sb[:, sc, :])

    nc.compile()
    _CACHE["nc"] = nc
    return nc


def kernel(x: np.ndarray, w_qkv: np.ndarray, w_o: np.ndarray) -> np.ndarray:
    import ml_dtypes
    from concourse.bass_utils import run_bass_kernel_spmd

    nc = _build()

    def ilv(a, dt):
        # [1024, n] -> [128, 8, n]: hidden index (kc*128 + p) -> (p, kc)
        return np.ascontiguousarray(
            a.reshape(8, P, -1).transpose(1, 0, 2)).astype(dt)

    f8 = ml_dtypes.float8_e4m3fn
    bf = ml_dtypes.bfloat16
    xT = [np.ascontiguousarray(x[b].T) for b in range(B)]
    xt8 = [ilv(t, f8) for t in xT]
    xtb = [ilv(t, bf) for t in xT]

    in_maps = []
    for c in range(N_CORES):
        b, g = divmod(c, 4)
        qcols = w_qkv[:, 256 * g:256 * g + 256]
        kcols = w_qkv[:, NH * HD + 256 * g:NH * HD + 256 * g + 256]
        vcols = w_qkv[:, 2 * NH * HD + 256 * g:2 * NH * HD + 256 * g + 256]
        wqk = np.concatenate([qcols, kcols], axis=1)  # [1024, 512]
        in_maps.append({
            "xt8": xt8[b],
            "xtb": xtb[b],
            "wqk8": ilv(wqk, f8),
            "wvb": ilv(vcols, bf),
            "wo": np.ascontiguousarray(w_o[256 * g:256 * g + 256, :]).astype(bf),
        })

    res = run_bass_kernel_spmd(nc, in_maps, list(range(N_CORES)),
                               **_CACHE.get("run_kwargs", {}))
    _CACHE["last_result"] = res

    out = np.zeros((B, S, HIDDEN), np.float32)
    for c in range(N_CORES):
        out[c // 4] += res.results[c]["out"]
    return out


# revision 11
# speedup vs baseline: 1.4181x; 1.4181x over previous
"""Trainium2 Bass kernel for nn_BaselineAttention_36172214567310.

Reference computation (note the einsum 'bhqk,bhkd->bhkd' bug: the "attention
output" is v scaled by the column-sums of the softmax matrix):

    qkv = x @ w_qkv                       # [b, s, 3*H*D]
    q, k, v = split(qkv)                  # per head
    P = softmax(q @ k^T / sqrt(D))        # [q, k] rows sum to 1
    colsum[k] = sum_q P[q, k]
    values[k, :] = v[k, :] * colsum_h[k]
    out = values @ w_o

Sharding: 8 cores = 2 batches x 4 head-groups (4 heads each).

Per-core pipeline. The ACT exp stream is the pacing engine (~183us:
128 x [128,1024] ACTIVATE + fused-rowsum drain); every other engine is
sized to stay off its critical path even if the PE is HAM-throttled to
1.2 GHz:
  - Q/K projection in fp8 DoubleRow (K=256 per matmul), V in bf16.
  - Scores bf16, K=64, head pair A/B row-tiled (lhsT base partitions 0/64)
    so the two heads' score matmuls run concurrently; A fills U0 while
    B fills U1, in [128,1024] halves ping-ponged against the exps.
  - exp on ACT -> fp8e4 E tiles in SBUF + accum_out rowsum per half.
  - colsum matvec: plain fp8 (K=128 per q-chunk), weights 1/rowsum (fp8e5)
    replicated over 64 stationary cols, heads col-tiled into a RESIDENT
    4-bank PSUM accumulator ps_c (A -> partitions 0-63, B -> 64-127),
    accumulated across all 16 q-chunks with no intermediate flushes.
    The first matmul of each pair uses a zero-padded [128,128] weight so
    its bank-wide has_written clear covers head B's region too.
  - v *= colsum directly from ps_c, then a tail phase computes
    out = values @ w_o with both head-pairs accumulated in PSUM, copies
    alternating between DVE and ACT, and chunk-streamed DMA out.
    Host sums the 4 group partials per batch.

PSUM: U0, U1 ([128,1024] f32, 2 banks each) + ps_c ([128,2048] f32,
4 banks) = all 8 banks. Projection leftovers borrow idle U windows.
"""

import sys

sys.path.insert(0, "/opt/trn_rl_repo")

import numpy as np

B, S, HIDDEN = 2, 2048, 1024
NH, HD = 16, 64
N_CORES = 8
P = 128
QC = S // P          # 16 q chunks of 128

_CACHE = {}


def _build():
    if "nc" in _CACHE:
        return _CACHE["nc"]

    import concourse.mybir as mybir
    import concourse.tile as tile
    from concourse import bacc

    F32 = mybir.dt.float32
    BF16 = mybir.dt.bfloat16
    F8E4 = mybir.dt.float8e4
    F8E5 = mybir.dt.float8e5
    EXP = mybir.ActivationFunctionType.Exp
    ADD = mybir.AluOpType.add
    MULT = mybir.AluOpType.mult
    DR = mybir.MatmulPerfMode.DoubleRow

    nc = bacc.Bacc()
    xt8_d = nc.declare_dram_parameter("xt8", [P, 8, S], F8E4, isOutput=False)
    xtb_d = nc.declare_dram_parameter("xtb", [P, 8, S], BF16, isOutput=False)
    wqk8_d = nc.declare_dram_parameter("wqk8", [P, 8, 512], F8E4, isOutput=False)
    wvb_d = nc.declare_dram_parameter("wvb", [P, 8, 256], BF16, isOutput=False)
    wo_d = nc.declare_dram_parameter("wo", [256, HIDDEN], BF16, isOutput=False)
    out_d = nc.declare_dram_parameter("out", [S, HIDDEN], F32, isOutput=True)

    with tile.TileContext(nc) as tc:
        with tc.tile_pool(name="sb", bufs=1) as sb, \
             tc.tile_pool(name="ps", bufs=1, space="PSUM") as ps:

            # ---- persistent SBUF tiles ----
            qt = [sb.tile([P, S], BF16, name=f"qt{g}") for g in range(2)]
            kt = [sb.tile([P, S], BF16, name=f"kt{g}") for g in range(2)]
            vt = [sb.tile([P, S], BF16, name=f"vt{g}") for g in range(2)]
            xt8 = sb.tile([P, 8, S], F8E4, name="xt8")
            xtb = sb.tile([P, 8, S], BF16, name="xtb")
            wqk8 = sb.tile([P, 8, 512], F8E4, name="wqk8")
            wvb = sb.tile([P, 8, 256], BF16, name="wvb")
            wo_t = [sb.tile([P, HIDDEN], BF16, name=f"wo{g}") for g in range(2)]
            # E + rowsums, ping-pong per head over qc parity
            e_t = [[sb.tile([P, S], F8E4, name=f"e{h}{i}") for i in range(2)]
                   for h in range(2)]
            r_t = [[sb.tile([P, 2], F32, name=f"r{h}{i}") for i in range(2)]
                   for h in range(2)]
            rs_t = [[sb.tile([P, 1], F32, name=f"rs{h}{i}") for i in range(2)]
                    for h in range(2)]
            wr_t = [[sb.tile([P, 64], F8E5, name=f"wr{h}{i}") for i in range(2)]
                    for h in range(2)]
            # zero-padded first-matvec weights (head A cols 0-63, zeros after)
            wr_pad = sb.tile([P, 128], F8E5, name="wr_pad")
            stage = [sb.tile([P, HIDDEN], F32, name=f"stage{i}") for i in range(2)]
            warm = sb.tile([P, 1], F32, name="warm")

            # PSUM: 2+2 banks of score halves, 4 banks of colsum accumulator
            U = [ps.tile([P, 1024], F32, name=f"U{i}") for i in range(2)]
            ps_c = ps.tile([P, S], F32, name="ps_c")

            # ---- exp table preload (overlaps input DMA) ----
            nc.vector.memset(warm, 0.0)
            nc.scalar.activation(warm, warm, EXP)
            nc.vector.memset(wr_pad[:, 64:128], 0.0)

            # ---- input DMA (Q/K path first: it gates the pipeline) ----
            nc.sync.dma_start(out=wqk8, in_=wqk8_d[:, :, :])
            nc.sync.dma_start(out=xt8, in_=xt8_d[:, :, :])
            for g in range(2):
                nc.sync.dma_start(out=wo_t[g], in_=wo_d[g * P:(g + 1) * P, :])
            nc.sync.dma_start(out=wvb, in_=wvb_d[:, :, :])
            nc.sync.dma_start(out=xtb, in_=xtb_d[:, :, :])

            PSLOT = [(U[0], 0), (U[0], 512), (U[1], 0), (U[1], 512),
                     (ps_c, 0), (ps_c, 512), (ps_c, 1024), (ps_c, 1536)]

            # wqk8 column layout: [Q01, Q23, K01, K23] x 128
            def qk_group(mcol, dst, nt, half, slot):
                """fp8 DoubleRow projection subgroup: 4 K=256 matmuls into a
                borrowed 256-col psum chunk, then f32->bf16 copy out."""
                tl, off = PSLOT[slot]
                pt = tl[:, off:off + 256]
                lo = nt * 512 + half * 256
                for j in range(4):
                    nc.tensor.matmul(
                        pt, wqk8[:, 2 * j:2 * j + 2, mcol * P:(mcol + 1) * P],
                        xt8[:, 2 * j:2 * j + 2, lo:lo + 256],
                        start=(j == 0), stop=(j == 3), perf_mode=DR)
                nc.vector.tensor_copy(out=dst[:, lo:lo + 256], in_=pt)

            def v_group(mc, nt, half, slot):
                """bf16 V projection subgroup: 8 K=128 matmuls, N=256."""
                tl, off = PSLOT[slot]
                pt = tl[:, off:off + 256]
                lo = nt * 512 + half * 256
                for kc in range(8):
                    nc.tensor.matmul(
                        pt, wvb[:, kc, mc * P:(mc + 1) * P],
                        xtb[:, kc, lo:lo + 256],
                        start=(kc == 0), stop=(kc == 7))
                nc.vector.tensor_copy(out=vt[mc][:, lo:lo + 256], in_=pt)

            def emit_scores_half(g, qc, hh):
                """Head A and B score matmuls for k-half hh, interleaved so
                they run concurrently in PE row groups 0-1 / 2-3."""
                for n in range(2):
                    ks = hh * 1024 + n * 512
                    for h in range(2):
                        bp = h * 64
                        nc.tensor.matmul(
                            U[h][:, n * 512:(n + 1) * 512],
                            qt[g][bp:bp + 64, qc * P:(qc + 1) * P],
                            kt[g][bp:bp + 64, ks:ks + 512],
                            start=True, stop=True)

            def emit_exp_half(h, t, hh):
                nc.scalar.activation(
                    e_t[h][t % 2][:, hh * 1024:(hh + 1) * 1024], U[h],
                    EXP, scale=0.125,
                    accum_out=r_t[h][t % 2][:, hh:hh + 1])

            def emit_wr(h, t):
                """1/rowsum matvec weights for chunk t of head h (fp8e5,
                replicated across 64 stationary cols; chunk 0 of a pair also
                fills the zero-padded wide weight for the bank-clear)."""
                i = t % 2
                nc.vector.tensor_tensor(rs_t[h][i], r_t[h][i][:, 0:1],
                                        r_t[h][i][:, 1:2], ADD)
                nc.vector.reciprocal(rs_t[h][i], rs_t[h][i])
                nc.vector.tensor_copy(
                    out=wr_t[h][i], in_=rs_t[h][i].to_broadcast([P, 64]))
                if h == 0 and t % QC == 0:
                    nc.vector.tensor_copy(
                        out=wr_pad[:, 0:64],
                        in_=rs_t[h][i].to_broadcast([P, 64]))

            def emit_mv(t):
                """Colsum matvec for chunk t: plain fp8 K=128, head A into
                ps_c partitions 0-63, head B into 64-127, accumulated in
                PSUM across the pair's 16 chunks."""
                qc = t % QC
                for c in range(4):
                    cs = slice(c * 512, (c + 1) * 512)
                    for h in range(2):
                        if h == 0 and qc == 0:
                            # wide zero-padded weights: bank-wide has_written
                            # clear covers head B's partitions as well
                            nc.tensor.matmul(
                                ps_c[:, cs], wr_pad, e_t[0][t % 2][:, cs],
                                start=True, stop=False, skip_group_check=True)
                        else:
                            nc.tensor.matmul(
                                ps_c[h * 64:h * 64 + 64, cs],
                                wr_t[h][t % 2], e_t[h][t % 2][:, cs],
                                start=False, stop=(qc == QC - 1 and h == 1),
                                skip_group_check=True)

            # ---- lead-in: K01/Q01 projection (gates first scores) ----
            for i, mcol in enumerate((2, 0)):
                dst = kt[0] if mcol == 2 else qt[0]
                for nt in range(4):
                    for half in range(2):
                        qk_group(mcol, dst, nt, half, slot=(i * 8 + nt * 2 + half) % 8)

            # Borrowed-window queue, ordered by deadline: V01 gates the
            # pair-0 v-scale (t=17); kt[1] and qt[1] nt0 gate pair-1 scores
            # (t=16); qt[1] ntX gates t=16+4X; V23 gates the pair-1 v-scale.
            # wqk8 mcols are [Q01, Q23, K01, K23]: Q23 = mcol 1, K23 = mcol 3.
            win = [(v_group, (0, nt, half)) for nt in range(4) for half in range(2)]
            win += [(qk_group, (1, qt[1], 0, half)) for half in range(2)]
            win += [(qk_group, (3, kt[1], nt, half))
                    for nt in range(4) for half in range(2)]
            win += [(qk_group, (1, qt[1], nt, half))
                    for nt in (1, 2, 3) for half in range(2)]
            win += [(v_group, (1, nt, half)) for nt in range(4) for half in range(2)]

            # ---- main loop: flat chunk stream t = g*16 + qc ----
            for t in range(2 * QC):
                g, qc = divmod(t, QC)
                # half 0: scores A/B then exps
                emit_scores_half(g, qc, 0)
                emit_exp_half(0, t, 0)
                emit_exp_half(1, t, 0)
                if t == QC + 1:
                    # pair-0 colsums complete: v *= colsum (before pair-1's
                    # first matvec clears ps_c below)
                    nc.vector.tensor_tensor(vt[0], vt[0], ps_c, MULT)
                # matvec for the previous chunk (weights built last iter)
                if t >= 1:
                    emit_mv(t - 1)
                # half 1
                emit_scores_half(g, qc, 1)
                emit_exp_half(0, t, 1)
                emit_exp_half(1, t, 1)
                for h in range(2):
                    emit_wr(h, t)
                # borrowed-window projection work
                if t >= 1 and win:
                    fn, args = win.pop(0)
                    fn(*args, slot=2 + (t % 2))      # U1 halves
                if t >= 13 and win:
                    fn, args = win.pop(0)
                    fn(*args, slot=t % 2)            # U0 halves
            # last chunk's matvec + scale
            emit_mv(2 * QC - 1)
            nc.vector.tensor_tensor(vt[1], vt[1], ps_c, MULT)

            # ---- tail: out = values @ w_o, both pairs PSUM-accumulated ----
            for sc in range(QC):
                st = stage[sc % 2]
                for nh in range(2):
                    tl, off = PSLOT[(sc * 2 + nh) % 8]
                    pt = tl[:, off:off + 512]
                    for g in range(2):
                        nc.tensor.matmul(
                            pt, vt[g][:, sc * P:(sc + 1) * P],
                            wo_t[g][:, nh * 512:(nh + 1) * 512],
                            start=(g == 0), stop=(g == 1))
                    dst = st[:, nh * 512:(nh + 1) * 512]
                    if nh == 0:
                        nc.vector.tensor_copy(out=dst, in_=pt)
                    else:
                        nc.scalar.copy(out=dst, in_=pt)
                nc.sync.dma_start(out=out_d[sc * P:(sc + 1) * P, :], in_=st)

    nc.compile()
    _CACHE["nc"] = nc
    return nc


def kernel(x: np.ndarray, w_qkv: np.ndarray, w_o: np.ndarray) -> np.ndarray:
    import ml_dtypes
    from concourse.bass_utils import run_bass_kernel_spmd

    nc = _build()

    def ilv(a, dt):
        # [1024, n] -> [128, 8, n]: hidden index (kc*128 + p) -> (p, kc)
        return np.ascontiguousarray(
            a.reshape(8, P, -1).transpose(1, 0, 2)).astype(dt)

    f8 = ml_dtypes.float8_e4m3fn
    bf = ml_dtypes.bfloat16
    xT = [np.ascontiguousarray(x[b].T) for b in range(B)]
    xt8 = [ilv(t, f8) for t in xT]
    xtb = [ilv(t, bf) for t in xT]

    in_maps = []
    for c in range(N_CORES):
        b, g = divmod(c, 4)
        qcols = w_qkv[:, 256 * g:256 * g + 256]
        kcols = w_qkv[:, NH * HD + 256 * g:NH * HD + 256 * g + 256]
        vcols = w_qkv[:, 2 * NH * HD + 256 * g:2 * NH * HD + 256 * g + 256]
        wqk = np.concatenate([qcols, kcols], axis=1)  # [1024, 512]
        in_maps.append({
            "xt8": xt8[b],
            "xtb": xtb[b],
            "wqk8": ilv(wqk, f8),
            "wvb": ilv(vcols, bf),
            "wo": np.ascontiguousarray(w_o[256 * g:256 * g + 256, :]).astype(bf),
        })

    res = run_bass_kernel_spmd(nc, in_maps, list(range(N_CORES)),
                               **_CACHE.get("run_kwargs", {}))
    _CACHE["last_result"] = res

    out = np.zeros((B, S, HIDDEN), np.float32)
    for c in range(N_CORES):
        out[c // 4] += res.results[c]["out"]
    return out
